# revision 14
# baseline (speedup 1.0000x reference)
"""Trainium2 Bass kernel for ChebyNet (K=1) forward pass.

ChebConv with K=1 reduces to a plain linear layer on the T0 (identity) term,
so edge_index / edge_weight never enter the math. The network is:

    h1 = x @ W1.T (+b1)           -> BN (train mode, over nodes) -> ReLU
    h2 = a1 @ W2.T (+b2)          -> BN -> ReLU
    h3 = relu(h2 @ Wl1.T + bl1)
    out = log_softmax(h3 @ Wl2.T + bl2, axis=1)

(b1/b2 cancel exactly inside train-mode BN and are dropped.)

Sharding: nodes (N=50000) split across 8 NeuronCores (6250 rows each).
Everything is computed feature-on-partition ([feat, rows]).

Design (vs the 516-593us v0 baseline):
 - All transposes / dtype packing on host: xT, x row-tiles (with a ones
   column for the column-sum), W1T, W2T/Wl1T/Wl2T, per-partition BN
   parameter columns. No on-device transposes.
 - bf16 operands everywhere on the matmul path (fp8 DoubleRow measured at
   rel_err 2.6-3.4e-2 for the K=1024 layers - over the 2e-2 gate).
 - BN1 stats analytically from the Gram matrix of x, projected locally to
   diag(W1 G W1^T) BEFORE the AllReduce -> payload [128,16] (8KB).
 - During the AR1 wait, L1 (pre-BN) is computed for all chunks into SBUF
   (bf16). The main pass applies BN1+ReLU, runs L2, and overwrites h1 with
   h2 *in place* in SBUF - h2 never spills to DRAM.
 - BN2 stats: sum(h2) via vector tensor_scalar accum_out during the
   PSUM->SBUF copy; sumsq(h2) split scalar(Square+accum)/vector(STT+accum).
 - log_softmax batched at the end ([10, R] feature-major) so the scalar
   activation table is not thrashed per chunk (Relu<->Exp<->Ln reloads cost
   1.3us each); -ln(sum) is broadcast via a K=1 matmul. Output is [10, R];
   host transposes to [R, 10] (layout-only, like the xT input).
"""

import os
import sys

sys.path.insert(0, "/opt/trn_rl_repo")

import numpy as np
import ml_dtypes

NCORES = 8
N_TOTAL = 50000
R = N_TOTAL // NCORES  # 6250 rows per core
DIN = 128
H = 1024
HM = 256
C = 10
BN_EPS = 1e-5

CH = 1024  # main-pass row chunk
FCH = 512  # final-pass row chunk
CH_LIST = [(i * CH, min(CH, R - i * CH)) for i in range((R + CH - 1) // CH)]
FCH_LIST = [(i * FCH, min(FCH, R - i * FCH)) for i in range((R + FCH - 1) // FCH)]
if os.environ.get("CH_LIMIT"):
    CH_LIST = CH_LIST[: int(os.environ["CH_LIMIT"])]
NCH = len(CH_LIST)

NRT = (R + 127) // 128  # 49 row tiles for the Gram matrix
D1 = DIN + 1  # x tile width incl the ones column

L2_FP8 = os.environ.get("L2_FP8", "0") == "1"
L3_FP8 = os.environ.get("L3_FP8", "0") == "1"

_CACHE = {}


def _halves(cc, step=512):
    out = []
    off = 0
    while off < cc:
        out.append((off, min(step, cc - off)))
        off += step
    return out


def _build(stage="full"):
    import concourse.bass as bass  # noqa: F401
    import concourse.tile as tile
    import concourse.mybir as mybir
    from concourse import bacc

    fp32 = mybir.dt.float32
    f32r = mybir.dt.float32r
    bf16 = mybir.dt.bfloat16
    fp8 = mybir.dt.float8e4
    AF = mybir.ActivationFunctionType
    ALU = mybir.AluOpType
    X = mybir.AxisListType.X
    DR = mybir.MatmulPerfMode.DoubleRow

    l2dt = fp8 if L2_FP8 else bf16
    l3dt = fp8 if L3_FP8 else bf16

    nc = bacc.Bacc(num_devices=NCORES, debug=False)

    x2_d = nc.dram_tensor("x2", [128, NRT * D1], bf16, kind="ExternalInput")
    xt_d = nc.dram_tensor("xT", [128, R], bf16, kind="ExternalInput")
    w1f_d = nc.dram_tensor("w1f", [128, H], f32r, kind="ExternalInput")
    w1b_d = nc.dram_tensor("w1b", [128, H], bf16, kind="ExternalInput")
    w2b_d = nc.dram_tensor("w2b", [128, 6 * H], bf16, kind="ExternalInput")
    w2f8_d = nc.dram_tensor("w2f8", [128, 2 * H], fp8, kind="ExternalInput")
    wl1p_d = nc.dram_tensor("wl1p", [128, 8 * HM], l3dt, kind="ExternalInput")
    wl2t_d = nc.dram_tensor("wl2t", [128, 2 * C], bf16, kind="ExternalInput")
    vc_d = nc.dram_tensor("vc", [128, 64], fp32, kind="ExternalInput")
    bl2c_d = nc.dram_tensor("bl2c", [16, 1], fp32, kind="ExternalInput")
    ones_d = nc.dram_tensor("ones", [128, 1], f32r, kind="ExternalInput")
    mones_d = nc.dram_tensor("mones", [1, 16], f32r, kind="ExternalInput")
    out_d = nc.dram_tensor("out", [C, R], fp32, kind="ExternalOutput")

    RG = [list(range(NCORES))]

    with tile.TileContext(nc) as tc:
        with (
            tc.tile_pool(name="persist", bufs=1) as persist,
            tc.tile_pool(name="dram", bufs=1, space="DRAM") as dram,
        ):
            # ---------------- persistent tiles -----------------
            hbuf = [
                persist.tile([128, R], bf16, tag=f"hb{m}", name=f"hbuf{m}")
                for m in range(8)
            ]
            w1bs = persist.tile([128, H], bf16, tag="w1bs", name="w1bs")
            w2bs = persist.tile([128, 6 * H], bf16, tag="w2bs", name="w2bs")
            w2f8s = persist.tile([128, 2 * H], fp8, tag="w2f8s", name="w2f8s")
            wl1ps = persist.tile([128, 8 * HM], l3dt, tag="wl1ps", name="wl1ps")
            wl2ts = persist.tile([128, 2 * C], bf16, tag="wl2ts", name="wl2ts")
            vcs = persist.tile([128, 64], fp32, tag="vcs", name="vcs")
            bl2cs = persist.tile([16, 1], fp32, tag="bl2cs", name="bl2cs")
            ones_r = persist.tile([128, 1], f32r, tag="ones_r", name="ones_r")
            mones10 = persist.tile([1, 16], f32r, tag="mones", name="mones10")
            bn1_s = persist.tile([128, 8], fp32, tag="bn1s", name="bn1_s")
            bn1_b = persist.tile([128, 8], fp32, tag="bn1b", name="bn1_b")
            bn2_s = persist.tile([128, 8], fp32, tag="bn2s", name="bn2_s")
            bn2_b = persist.tile([128, 8], fp32, tag="bn2b", name="bn2_b")
            eps_c = persist.tile([128, 1], fp32, tag="epsc", name="eps_c")
            suma = persist.tile([128, 8 * NCH], fp32, tag="suma", name="suma")
            sumq = persist.tile([128, 8 * NCH], fp32, tag="sumq", name="sumq")
            st1g = persist.tile([128, 16], fp32, tag="st1g", name="st1g")
            st2g = persist.tile([128, 16], fp32, tag="st2g", name="st2g")
            ones_bf = persist.tile([128, 1], bf16, tag="onesbf", name="ones_bf")
            mones_bf = persist.tile([1, 16], bf16, tag="monesbf", name="mones_bf")
            lgall = persist.tile([16, R], bf16, tag="lgall", name="lgall")
            e_all = persist.tile([16, R], bf16, tag="e_all", name="e_all")

            cc1_in = dram.tile([128, 16], fp32, name="cc1_in")
            cc1_out = dram.tile([128, 16], fp32, name="cc1_out")
            cc2_in = dram.tile([128, 16], fp32, name="cc2_in")
            cc2_out = dram.tile([128, 16], fp32, name="cc2_out")

            nc.vector.memset(eps_c[:], BN_EPS)
            nc.vector.memset(ones_bf[:], 1.0)
            nc.vector.memset(mones_bf[:], -1.0)
            nc.scalar.dma_start(out=ones_r[:], in_=ones_d[:])
            nc.scalar.dma_start(out=mones10[:], in_=mones_d[:])

            # per-partition views of the BN parameter columns
            vcv = vcs[:].rearrange("p (m j) -> p j m", j=8)  # [128, j, m]

            # ============ startup + prefill (xT lives only here) ============
            with tc.tile_pool(name="xtp", bufs=1) as xtp:
                xTs = xtp.tile([128, R], bf16, tag="xTs", name="xTs")

                with (
                    tc.tile_pool(name="startsb", bufs=1) as startsb,
                    tc.tile_pool(name="startps", bufs=1, space="PSUM") as startps,
                ):
                    x2s = startsb.tile(
                        [128, NRT * D1], bf16, tag="x2s", name="x2s"
                    )
                    w1fr = startsb.tile([128, H], f32r, tag="w1fr", name="w1fr")
                    v_r = startsb.tile([128, H], bf16, tag="v_r", name="v_r")
                    mean_r = startsb.tile([128, 1], bf16, tag="mean_r", name="mean_r")
                    st1sb = startsb.tile([128, 16], fp32, tag="st1sb", name="st1sb")

                    # big loads on the sync DMA queue; small ones on scalar
                    nc.sync.dma_start(out=x2s[:], in_=x2_d[:])
                    nc.sync.dma_start(out=xTs[:], in_=xt_d[:])
                    nc.sync.dma_start(out=w2bs[:], in_=w2b_d[:])
                    nc.sync.dma_start(out=w2f8s[:], in_=w2f8_d[:])
                    nc.sync.dma_start(out=wl1ps[:], in_=wl1p_d[:])
                    nc.scalar.dma_start(out=w1fr[:], in_=w1f_d[:])
                    nc.scalar.dma_start(out=w1bs[:], in_=w1b_d[:])
                    nc.scalar.dma_start(out=vcs[:], in_=vc_d[:])
                    nc.scalar.dma_start(out=wl2ts[:], in_=wl2t_d[:])
                    nc.scalar.dma_start(out=bl2cs[:], in_=bl2c_d[:])

                    # Gram matrix of x (incl ones column -> column sums)
                    gram_ps = startps.tile(
                        [128, D1], fp32, tag="gram", name="gram_ps"
                    )
                    for t in range(NRT):
                        o = t * D1
                        nc.tensor.matmul(
                            gram_ps[:],
                            lhsT=x2s[:, o : o + DIN],
                            rhs=x2s[:, o : o + D1],
                            start=(t == 0),
                            stop=(t == NRT - 1),
                        )
                    gram_r = startsb.tile(
                        [128, D1], f32r, tag="gram_r", name="gram_r"
                    )
                    nc.vector.tensor_copy(gram_r[:], gram_ps[:])
                    nc.scalar.mul(mean_r[:], gram_r[:, DIN : D1], 1.0 / N_TOTAL)

                    # P = G @ W1T ; V = W1T*P ; e2[f]=colsum(V) ; wxm = W1T.T mean
                    st1_ps = startps.tile(
                        [128, 16], fp32, tag="st1ps", name="st1_ps"
                    )
                    for hf in range(2):
                        sl = slice(hf * 512, (hf + 1) * 512)
                        p_ps = startps.tile(
                            [128, 512], fp32, tag=f"pps{hf}", name=f"p_ps{hf}"
                        )
                        nc.tensor.matmul(
                            p_ps[:], lhsT=gram_r[:, 0:DIN], rhs=w1fr[:, sl],
                            start=True, stop=True,
                        )
                        nc.vector.tensor_mul(v_r[:, sl], w1fr[:, sl], p_ps[:])
                    for m in range(8):
                        sl = slice(m * 128, (m + 1) * 128)
                        nc.tensor.matmul(
                            st1_ps[:, m : m + 1], lhsT=v_r[:, sl], rhs=ones_bf[:],
                            start=True, stop=True,
                        )
                        nc.tensor.matmul(
                            st1_ps[:, 8 + m : 9 + m], lhsT=w1bs[:, sl],
                            rhs=mean_r[:], start=True, stop=True,
                        )
                    nc.vector.tensor_copy(st1sb[:], st1_ps[:])
                    nc.sync.dma_start(out=cc1_in[:], in_=st1sb[:])
                    nc.gpsimd.collective_compute(
                        "AllReduce", ALU.add, replica_groups=RG,
                        ins=[cc1_in[:].opt()], outs=[cc1_out[:].opt()],
                    )
                    nc.sync.dma_start(out=st1g[:], in_=cc1_out[:])

                # -------- L1 prefill (runs during the AllReduce wait) -------
                with tc.tile_pool(name="ph1", bufs=4, space="PSUM") as ph1:
                    for ci, (c0, cc) in enumerate(CH_LIST):
                        for m in range(8):
                            hp = ph1.tile(
                                [128, CH], fp32, tag="ph1", name=f"h1_{ci}_{m}"
                            )
                            for off, nn in _halves(cc):
                                nc.tensor.matmul(
                                    hp[:, off : off + nn],
                                    lhsT=w1bs[:, m * 128 : (m + 1) * 128],
                                    rhs=xTs[:, c0 + off : c0 + off + nn],
                                    start=True,
                                    stop=True,
                                )
                            if m % 2 == 0:
                                nc.scalar.copy(
                                    hbuf[m][:, c0 : c0 + cc], hp[:, :cc]
                                )
                            else:
                                nc.vector.tensor_copy(
                                    hbuf[m][:, c0 : c0 + cc], hp[:, :cc]
                                )

            # ---------------- BN1 parameters (vectorized) ----------------
            with tc.tile_pool(name="bnw", bufs=1) as bnw:
                var8 = bnw.tile([128, 8], fp32, tag="v8", name="var8")
                msq8 = bnw.tile([128, 8], fp32, tag="m8", name="msq8")
                sd8 = bnw.tile([128, 8], fp32, tag="s8", name="sd8")
                rstd8 = bnw.tile([128, 8], fp32, tag="r8", name="rstd8")
                t8 = bnw.tile([128, 8], fp32, tag="t8", name="t8")
                nc.vector.tensor_scalar_mul(var8[:], st1g[:, 0:8], 1.0 / N_TOTAL)
                nc.vector.tensor_mul(msq8[:], st1g[:, 8:16], st1g[:, 8:16])
                nc.vector.tensor_sub(var8[:], var8[:], msq8[:])
                nc.scalar.activation(sd8[:], var8[:], AF.Sqrt, bias=eps_c[:])
                nc.vector.reciprocal(rstd8[:], sd8[:])
                nc.vector.tensor_mul(bn1_s[:], rstd8[:], vcv[:, 1, :])
                nc.vector.tensor_mul(t8[:], st1g[:, 8:16], bn1_s[:])
                nc.vector.tensor_sub(bn1_b[:], vcv[:, 2, :], t8[:])

            if stage == "s1":
                dummy = persist.tile([16, R], fp32, tag="dummy", name="dummy")
                nc.vector.memset(dummy[:], 0.0)
                nc.vector.tensor_copy(dummy[:10, 0:8], bn1_s[:10, :])
                nc.vector.tensor_copy(dummy[:10, 8:16], bn1_b[:10, :])
                nc.sync.dma_start(out=out_d[:], in_=dummy[:10, :])
                nc.finalize()
                return nc

            # ------------- main pass: BN1+ReLU -> L2 -> stats ------------
            w2bv = w2bs[:].rearrange("p (k h) -> p k h", k=6)
            w2f8v = w2f8s[:].rearrange("p (two h) -> p two h", two=2)
            with (
                tc.tile_pool(name="acts", bufs=1) as acts,
                tc.tile_pool(name="sqp", bufs=1) as sqp,
                tc.tile_pool(name="ph2", bufs=3, space="PSUM") as ph2,
            ):
                for ci, (c0, cc) in enumerate(CH_LIST):
                    sl = slice(c0, c0 + cc)
                    a1t = [
                        acts.tile(
                            [128, CH], bf16, tag=f"act{ci & 1}_{k}",
                            name=f"a1_{ci}_{k}",
                        )
                        for k in range(6)
                    ]
                    a1f8 = acts.tile(
                        [128, 2 * CH], fp8, tag=f"actf8{ci & 1}", name=f"a1f8_{ci}"
                    )
                    a1f8v = a1f8[:].rearrange("p (two c) -> p two c", two=2)
                    for m in range(8):
                        dst = (
                            a1t[m][:, :cc]
                            if m < 6
                            else a1f8[:, (m - 6) * CH : (m - 6) * CH + cc]
                        )
                        nc.scalar.activation(
                            dst,
                            hbuf[m][:, sl],
                            AF.Relu,
                            bias=bn1_b[:, m : m + 1],
                            scale=bn1_s[:, m : m + 1],
                        )
                    for m2 in range(8):
                        hp = ph2.tile([128, CH], fp32, tag="ph2", name=f"h2_{ci}_{m2}")
                        msl = slice(m2 * 128, (m2 + 1) * 128)
                        for k in range(6):
                            for off, nn in _halves(cc):
                                nc.tensor.matmul(
                                    hp[:, off : off + nn],
                                    lhsT=w2bv[:, k, msl],
                                    rhs=a1t[k][:, off : off + nn],
                                    start=(k == 0),
                                    stop=False,
                                )
                        for off, nn in _halves(cc):
                            nc.tensor.matmul(
                                hp[:, off : off + nn],
                                lhsT=w2f8v[:, :, msl],
                                rhs=a1f8v[:, :, off : off + nn],
                                start=False,
                                stop=True,
                                perf_mode=DR,
                            )
                        # PSUM -> SBUF (bf16, in place over h1) + row-sum
                        nc.vector.tensor_scalar(
                            out=hbuf[m2][:, sl],
                            in0=hp[:, :cc],
                            scalar1=1.0,
                            scalar2=0.0,
                            op0=ALU.mult,
                            op1=ALU.add,
                            accum_out=suma[:, m2 * NCH + ci : m2 * NCH + ci + 1],
                        )
                        sq = sqp.tile(
                            [128, CH], bf16, tag=f"sq{m2 & 1}", name=f"sq_{ci}_{m2}"
                        )
                        if m2 < 3:
                            nc.scalar.activation(
                                sq[:, :cc],
                                hbuf[m2][:, sl],
                                AF.Square,
                                accum_out=sumq[:, m2 * NCH + ci : m2 * NCH + ci + 1],
                            )
                        else:
                            nc.vector.scalar_tensor_tensor(
                                out=sq[:, :cc],
                                in0=hbuf[m2][:, sl],
                                scalar=1.0,
                                in1=hbuf[m2][:, sl],
                                op0=ALU.mult,
                                op1=ALU.mult,
                                accum_out=sumq[:, m2 * NCH + ci : m2 * NCH + ci + 1],
                            )

                # ---------------- BN2 statistics ----------------
                st2sb = sqp.tile([128, 16], fp32, tag="st2sb", name="st2sb")
                sumav = suma[:].rearrange("p (m c) -> p m c", m=8)
                sumqv = sumq[:].rearrange("p (m c) -> p m c", m=8)
                nc.vector.reduce_sum(st2sb[:, 0:8], sumav, axis=X)
                nc.vector.reduce_sum(st2sb[:, 8:16], sumqv, axis=X)
                nc.sync.dma_start(out=cc2_in[:], in_=st2sb[:])
                nc.gpsimd.collective_compute(
                    "AllReduce", ALU.add, replica_groups=RG,
                    ins=[cc2_in[:].opt()], outs=[cc2_out[:].opt()],
                )
                nc.sync.dma_start(out=st2g[:], in_=cc2_out[:])

            # ---------------- BN2 parameters (vectorized) ----------------
            with tc.tile_pool(name="bnw2", bufs=1) as bnw2:
                mean8 = bnw2.tile([128, 8], fp32, tag="n8", name="mean8")
                var8b = bnw2.tile([128, 8], fp32, tag="v8", name="var8b")
                msq8b = bnw2.tile([128, 8], fp32, tag="m8", name="msq8b")
                sd8b = bnw2.tile([128, 8], fp32, tag="s8", name="sd8b")
                rstd8b = bnw2.tile([128, 8], fp32, tag="r8", name="rstd8b")
                t8b = bnw2.tile([128, 8], fp32, tag="t8", name="t8b")
                nc.vector.tensor_scalar_mul(mean8[:], st2g[:, 0:8], 1.0 / N_TOTAL)
                nc.vector.tensor_mul(msq8b[:], mean8[:], mean8[:])
                nc.vector.scalar_tensor_tensor(
                    out=var8b[:], in0=st2g[:, 8:16], scalar=1.0 / N_TOTAL,
                    in1=msq8b[:], op0=ALU.mult, op1=ALU.subtract,
                )
                nc.scalar.activation(sd8b[:], var8b[:], AF.Sqrt, bias=eps_c[:])
                nc.vector.reciprocal(rstd8b[:], sd8b[:])
                nc.vector.tensor_mul(bn2_s[:], rstd8b[:], vcv[:, 4, :])
                nc.vector.tensor_mul(t8b[:], mean8[:], bn2_s[:])
                nc.vector.tensor_sub(bn2_b[:], vcv[:, 5, :], t8b[:])

            # ------ final pass A: BN2+ReLU -> L3 -> L4 -> logits ------
            wl1v = wl1ps[:].rearrange("p (kp two h) -> p kp two h", kp=4, two=2)
            with (
                tc.tile_pool(name="acts2", bufs=1) as acts2,
                tc.tile_pool(name="fsb", bufs=2) as fsb,
                tc.tile_pool(name="ph3", bufs=3, space="PSUM") as ph3,
                tc.tile_pool(name="plg", bufs=2, space="PSUM") as plg,
            ):
                for ci, (c0, cc) in enumerate(FCH_LIST):
                    sl = slice(c0, c0 + cc)
                    a2t = [
                        acts2.tile(
                            [128, FCH], l3dt, tag=f"act{ci & 1}_{k}",
                            name=f"a2_{ci}_{k}",
                        )
                        for k in range(8)
                    ]
                    for m in range(8):
                        if m < 5:
                            nc.scalar.activation(
                                a2t[m][:, :cc],
                                hbuf[m][:, sl],
                                AF.Relu,
                                bias=bn2_b[:, m : m + 1],
                                scale=bn2_s[:, m : m + 1],
                            )
                        else:
                            tmp = fsb.tile(
                                [128, FCH], bf16, tag=f"tmp{m}", name=f"tmp_{ci}_{m}"
                            )
                            nc.vector.tensor_scalar(
                                out=tmp[:, :cc],
                                in0=hbuf[m][:, sl],
                                scalar1=bn2_s[:, m : m + 1],
                                scalar2=bn2_b[:, m : m + 1],
                                op0=ALU.mult,
                                op1=ALU.add,
                            )
                            nc.vector.tensor_scalar_max(
                                a2t[m][:, :cc], tmp[:, :cc], 0.0
                            )
                    h3t = []
                    for m3 in range(2):
                        hp3 = ph3.tile(
                            [128, FCH], fp32, tag="ph3", name=f"h3_{ci}_{m3}"
                        )
                        msl = slice(m3 * 128, (m3 + 1) * 128)
                        for k in range(8):
                            nc.tensor.matmul(
                                hp3[:, :cc],
                                lhsT=wl1v[:, k // 2, k % 2, msl],
                                rhs=a2t[k][:, :cc],
                                start=(k == 0),
                                stop=(k == 7),
                            )
                        h3 = fsb.tile(
                            [128, FCH], bf16, tag=f"h3_{m3}", name=f"h3t_{ci}_{m3}"
                        )
                        nc.vector.tensor_scalar(
                            out=h3[:, :cc],
                            in0=hp3[:, :cc],
                            scalar1=vcv[:, 6, m3 : m3 + 1],
                            scalar2=0.0,
                            op0=ALU.add,
                            op1=ALU.max,
                        )
                        h3t.append(h3)
                    # L4: logits (pre-bl2) -> PSUM -> lgall (bf16)
                    lg = plg.tile([C, FCH], fp32, tag="lg", name=f"lg_{ci}")
                    for k in range(2):
                        nc.tensor.matmul(
                            lg[:, :cc],
                            lhsT=wl2ts[:, k * C : (k + 1) * C],
                            rhs=h3t[k][:, :cc],
                            start=(k == 0),
                            stop=(k == 1),
                        )
                    nc.vector.tensor_copy(lgall[:C, sl], lg[:, :cc])

            # ---------- batched log_softmax over [10, R] ----------
            with (
                tc.tile_pool(name="smx", bufs=2) as smx,
                tc.tile_pool(name="psmB", bufs=2, space="PSUM") as psmB,
            ):
                EW = 2048
                for o0 in range(0, R, EW):
                    ee = min(EW, R - o0)
                    nc.scalar.activation(
                        e_all[:C, o0 : o0 + ee],
                        lgall[:C, o0 : o0 + ee],
                        AF.Exp,
                        bias=bl2cs[:C, :],
                    )
                NF = len(FCH_LIST)
                lns_t = {}
                for step in range(NF + 1):
                    if step < NF:
                        c0, cc = FCH_LIST[step]
                        sm = psmB.tile(
                            [1, FCH], fp32, tag=f"sm{step & 1}", name=f"smB_{step}"
                        )
                        nc.tensor.matmul(
                            sm[:, :cc],
                            lhsT=ones_bf[:C, :],
                            rhs=e_all[:C, c0 : c0 + cc],
                            start=True,
                            stop=True,
                        )
                        lns = smx.tile(
                            [1, FCH], bf16, tag=f"lns{step & 1}", name=f"lnsB_{step}"
                        )
                        nc.scalar.activation(lns[:, :cc], sm[:, :cc], AF.Ln)
                        lns_t[step] = lns
                    if step >= 1:
                        ci = step - 1
                        c0, cc = FCH_LIST[ci]
                        sl = slice(c0, c0 + cc)
                        bc = psmB.tile(
                            [C, FCH], fp32, tag=f"bc{ci & 1}", name=f"bcB_{ci}"
                        )
                        nc.tensor.matmul(
                            bc[:, :cc],
                            lhsT=mones_bf[:, :C],
                            rhs=lns_t[ci][:, :cc],
                            start=True,
                            stop=True,
                        )
                        osb = smx.tile(
                            [C, FCH], fp32, tag=f"osb{ci & 1}", name=f"osbB_{ci}"
                        )
                        nc.vector.scalar_tensor_tensor(
                            out=osb[:, :cc],
                            in0=lgall[:C, sl],
                            scalar=bl2cs[:C, :],
                            in1=bc[:, :cc],
                            op0=ALU.add,
                            op1=ALU.add,
                        )
                        nc.sync.dma_start(out=out_d[:, sl], in_=osb[:, :cc])

    nc.finalize()
    return nc


def _get_nc():
    if "nc" not in _CACHE:
        _CACHE["nc"] = _build(os.environ.get("KERNEL_STAGE", "full"))
    return _CACHE["nc"]


def make_in_maps(inputs):
    """Host-side layout/dtype prep. Layout-only transforms (transpose, tile,
    pack, cast) - all math happens on device."""
    f32 = np.float32
    bf = ml_dtypes.bfloat16
    f8 = ml_dtypes.float8_e4m3

    x = np.ascontiguousarray(np.asarray(inputs["x"]), dtype=f32)
    W1 = np.asarray(inputs["W1"], dtype=f32)
    W2 = np.asarray(inputs["W2"], dtype=f32)
    Wl1 = np.asarray(inputs["Wl1"], dtype=f32)
    Wl2 = np.asarray(inputs["Wl2"], dtype=f32)

    w1f = np.ascontiguousarray(W1.T)  # [128, H] f32 (f32r on device)
    w1b = w1f.astype(bf)
    l3np = f8 if L3_FP8 else bf
    # L2: first 768 contraction dims bf16, last 256 packed fp8 (DoubleRow)
    w2b = np.ascontiguousarray(
        W2.T[:768].reshape(6, 128, H).transpose(1, 0, 2).reshape(128, 6 * H)
    ).astype(bf)
    w2f8 = np.ascontiguousarray(
        W2.T[768:].reshape(2, 128, H).transpose(1, 0, 2).reshape(128, 2 * H)
    ).astype(f8)
    wl1p = np.ascontiguousarray(
        Wl1.T.reshape(4, 2, 128, HM).transpose(2, 0, 1, 3).reshape(128, 8 * HM)
    ).astype(l3np)
    wl2t = np.ascontiguousarray(
        Wl2.T.reshape(2, 128, C).transpose(1, 0, 2).reshape(128, 2 * C)
    ).astype(bf)

    vecs = np.zeros((8, H), f32)
    vecs[1, :] = np.asarray(inputs["g1"], dtype=f32)
    vecs[2, :] = np.asarray(inputs["be1"], dtype=f32)
    vecs[4, :] = np.asarray(inputs["g2"], dtype=f32)
    vecs[5, :] = np.asarray(inputs["be2"], dtype=f32)
    vecs[6, :HM] = np.asarray(inputs["bl1"], dtype=f32)
    # vc[p, k, j] = vecs[j, k*128+p]
    vc = np.ascontiguousarray(
        vecs.T.reshape(8, 128, 8).transpose(1, 0, 2).reshape(128, 64)
    )
    bl2c = np.zeros((16, 1), f32)
    bl2c[:C, 0] = np.asarray(inputs["bl2"], dtype=f32)

    in_maps = []
    for i in range(NCORES):
        xs = x[i * R : (i + 1) * R]
        xt = np.ascontiguousarray(xs.T).astype(bf)
        x2 = np.zeros((NRT * 128, D1), f32)
        x2[:R, :DIN] = xs
        x2[:R, DIN] = 1.0
        x2 = np.ascontiguousarray(
            x2.reshape(NRT, 128, D1).transpose(1, 0, 2).reshape(128, NRT * D1)
        ).astype(bf)
        in_maps.append(
            {
                "x2": x2, "xT": xt, "w1f": w1f, "w1b": w1b,
                "w2b": w2b, "w2f8": w2f8,
                "wl1p": wl1p, "wl2t": wl2t, "vc": vc, "bl2c": bl2c,
                "ones": np.ones((128, 1), f32),
                "mones": np.full((1, 16), -1.0, f32),
            }
        )
    return in_maps


def kernel(**inputs):
    from concourse.bass_utils import run_bass_kernel_spmd

    nc = _get_nc()
    in_maps = make_in_maps(inputs)
    res = run_bass_kernel_spmd(nc, in_maps, core_ids=list(range(NCORES)))
    return np.concatenate(
        [np.asarray(r["out"], dtype=np.float32).T for r in res.results], axis=0
    )


# revision 15
# speedup vs baseline: 1.0192x; 1.0192x over previous
"""Trainium2 Bass kernel for ChebyNet (K=1) forward pass.

ChebConv with K=1 reduces to a plain linear layer on the T0 (identity) term,
so edge_index / edge_weight never enter the math. The network is:

    h1 = x @ W1.T (+b1)           -> BN (train mode, over nodes) -> ReLU
    h2 = a1 @ W2.T (+b2)          -> BN -> ReLU
    h3 = relu(h2 @ Wl1.T + bl1)
    out = log_softmax(h3 @ Wl2.T + bl2, axis=1)

(b1/b2 cancel exactly inside train-mode BN and are dropped.)

Sharding: nodes (N=50000) split across 8 NeuronCores (6250 rows each).
Everything is computed feature-on-partition ([feat, rows]).

Design (vs the 516-593us v0 baseline):
 - All transposes / dtype packing on host: xT, x row-tiles (with a ones
   column for the column-sum), W1T, W2T/Wl1T/Wl2T, per-partition BN
   parameter columns. No on-device transposes.
 - bf16 operands everywhere on the matmul path (fp8 DoubleRow measured at
   rel_err 2.6-3.4e-2 for the K=1024 layers - over the 2e-2 gate).
 - BN1 stats analytically from the Gram matrix of x, projected locally to
   diag(W1 G W1^T) BEFORE the AllReduce -> payload [128,16] (8KB).
 - During the AR1 wait, L1 (pre-BN) is computed for all chunks into SBUF
   (bf16). The main pass applies BN1+ReLU, runs L2, and overwrites h1 with
   h2 *in place* in SBUF - h2 never spills to DRAM.
 - BN2 stats: sum(h2) via vector tensor_scalar accum_out during the
   PSUM->SBUF copy; sumsq(h2) split scalar(Square+accum)/vector(STT+accum).
 - log_softmax batched at the end ([10, R] feature-major) so the scalar
   activation table is not thrashed per chunk (Relu<->Exp<->Ln reloads cost
   1.3us each); -ln(sum) is broadcast via a K=1 matmul. Output is [10, R];
   host transposes to [R, 10] (layout-only, like the xT input).
"""

import os
import sys

sys.path.insert(0, "/opt/trn_rl_repo")

import numpy as np
import ml_dtypes

NCORES = 8
N_TOTAL = 50000
R = N_TOTAL // NCORES  # 6250 rows per core
DIN = 128
H = 1024
HM = 256
C = 10
BN_EPS = 1e-5

CH = 1024  # main-pass row chunk
FCH = 512  # final-pass row chunk
CH_LIST = [(i * CH, min(CH, R - i * CH)) for i in range((R + CH - 1) // CH)]
FCH_LIST = [(i * FCH, min(FCH, R - i * FCH)) for i in range((R + FCH - 1) // FCH)]
if os.environ.get("CH_LIMIT"):
    CH_LIST = CH_LIST[: int(os.environ["CH_LIMIT"])]
NCH = len(CH_LIST)

NRT = (R + 127) // 128  # 49 row tiles for the Gram matrix
D1 = DIN + 1  # x tile width incl the ones column

L2_FP8 = os.environ.get("L2_FP8", "0") == "1"
L3_FP8 = os.environ.get("L3_FP8", "0") == "1"

_CACHE = {}


def _halves(cc, step=512):
    out = []
    off = 0
    while off < cc:
        out.append((off, min(step, cc - off)))
        off += step
    return out


def _build(stage="full"):
    import concourse.bass as bass  # noqa: F401
    import concourse.tile as tile
    import concourse.mybir as mybir
    from concourse import bacc

    fp32 = mybir.dt.float32
    f32r = mybir.dt.float32r
    bf16 = mybir.dt.bfloat16
    fp8 = mybir.dt.float8e4
    AF = mybir.ActivationFunctionType
    ALU = mybir.AluOpType
    X = mybir.AxisListType.X
    DR = mybir.MatmulPerfMode.DoubleRow

    l2dt = fp8 if L2_FP8 else bf16
    l3dt = fp8 if L3_FP8 else bf16

    nc = bacc.Bacc(num_devices=NCORES, debug=False)

    x2_d = nc.dram_tensor("x2", [128, NRT * D1], bf16, kind="ExternalInput")
    xt_d = nc.dram_tensor("xT", [128, R], bf16, kind="ExternalInput")
    w1f_d = nc.dram_tensor("w1f", [128, H], f32r, kind="ExternalInput")
    w1b_d = nc.dram_tensor("w1b", [128, H], bf16, kind="ExternalInput")
    w2b_d = nc.dram_tensor("w2b", [128, 6 * H], bf16, kind="ExternalInput")
    w2f8_d = nc.dram_tensor("w2f8", [128, 2 * H], fp8, kind="ExternalInput")
    wl1p_d = nc.dram_tensor("wl1p", [128, 8 * HM], l3dt, kind="ExternalInput")
    wl2t_d = nc.dram_tensor("wl2t", [128, 2 * C], bf16, kind="ExternalInput")
    vc_d = nc.dram_tensor("vc", [128, 64], fp32, kind="ExternalInput")
    bl2c_d = nc.dram_tensor("bl2c", [16, 1], fp32, kind="ExternalInput")
    ones_d = nc.dram_tensor("ones", [128, 1], f32r, kind="ExternalInput")
    mones_d = nc.dram_tensor("mones", [1, 16], f32r, kind="ExternalInput")
    out_d = nc.dram_tensor("out", [C, R], fp32, kind="ExternalOutput")

    RG = [list(range(NCORES))]

    with tile.TileContext(nc) as tc:
        with (
            tc.tile_pool(name="persist", bufs=1) as persist,
            tc.tile_pool(name="dram", bufs=1, space="DRAM") as dram,
        ):
            # ---------------- persistent tiles -----------------
            hbuf = [
                persist.tile([128, R], bf16, tag=f"hb{m}", name=f"hbuf{m}")
                for m in range(8)
            ]
            w1bs = persist.tile([128, H], bf16, tag="w1bs", name="w1bs")
            w2bs = persist.tile([128, 6 * H], bf16, tag="w2bs", name="w2bs")
            w2f8s = persist.tile([128, 2 * H], fp8, tag="w2f8s", name="w2f8s")
            wl1ps = persist.tile([128, 8 * HM], l3dt, tag="wl1ps", name="wl1ps")
            wl2ts = persist.tile([128, 2 * C], bf16, tag="wl2ts", name="wl2ts")
            vcs = persist.tile([128, 64], fp32, tag="vcs", name="vcs")
            bl2cs = persist.tile([16, 1], fp32, tag="bl2cs", name="bl2cs")
            ones_r = persist.tile([128, 1], f32r, tag="ones_r", name="ones_r")
            mones10 = persist.tile([1, 16], f32r, tag="mones", name="mones10")
            bn1_s = persist.tile([128, 8], fp32, tag="bn1s", name="bn1_s")
            bn1_b = persist.tile([128, 8], fp32, tag="bn1b", name="bn1_b")
            bn2_s = persist.tile([128, 8], fp32, tag="bn2s", name="bn2_s")
            bn2_b = persist.tile([128, 8], fp32, tag="bn2b", name="bn2_b")
            eps_c = persist.tile([128, 1], fp32, tag="epsc", name="eps_c")
            suma = persist.tile([128, 8 * NCH], fp32, tag="suma", name="suma")
            sumq = persist.tile([128, 8 * NCH], fp32, tag="sumq", name="sumq")
            st1g = persist.tile([128, 16], fp32, tag="st1g", name="st1g")
            st2g = persist.tile([128, 16], fp32, tag="st2g", name="st2g")
            ones_bf = persist.tile([128, 1], bf16, tag="onesbf", name="ones_bf")
            mones_bf = persist.tile([1, 16], bf16, tag="monesbf", name="mones_bf")
            lgall = persist.tile([16, R], bf16, tag="lgall", name="lgall")
            e_all = persist.tile([16, R], bf16, tag="e_all", name="e_all")

            cc1_in = dram.tile([128, 16], fp32, name="cc1_in")
            cc1_out = dram.tile([128, 16], fp32, name="cc1_out")
            cc2_in = dram.tile([128, 16], fp32, name="cc2_in")
            cc2_out = dram.tile([128, 16], fp32, name="cc2_out")

            nc.vector.memset(eps_c[:], BN_EPS)
            nc.vector.memset(ones_bf[:], 1.0)
            nc.vector.memset(mones_bf[:], -1.0)
            nc.scalar.dma_start(out=ones_r[:], in_=ones_d[:])
            nc.scalar.dma_start(out=mones10[:], in_=mones_d[:])

            # per-partition views of the BN parameter columns
            vcv = vcs[:].rearrange("p (m j) -> p j m", j=8)  # [128, j, m]

            # ============ startup + prefill (xT lives only here) ============
            with tc.tile_pool(name="xtp", bufs=1) as xtp:
                xTs = xtp.tile([128, R], bf16, tag="xTs", name="xTs")

                with (
                    tc.tile_pool(name="startsb", bufs=1) as startsb,
                    tc.tile_pool(name="startps", bufs=1, space="PSUM") as startps,
                ):
                    x2s = startsb.tile(
                        [128, NRT * D1], bf16, tag="x2s", name="x2s"
                    )
                    w1fr = startsb.tile([128, H], f32r, tag="w1fr", name="w1fr")
                    v_r = startsb.tile([128, H], bf16, tag="v_r", name="v_r")
                    mean_r = startsb.tile([128, 1], bf16, tag="mean_r", name="mean_r")
                    st1sb = startsb.tile([128, 16], fp32, tag="st1sb", name="st1sb")

                    # big loads on the sync DMA queue; small ones on scalar
                    nc.sync.dma_start(out=x2s[:], in_=x2_d[:])
                    nc.sync.dma_start(out=xTs[:], in_=xt_d[:])
                    nc.sync.dma_start(out=w2bs[:], in_=w2b_d[:])
                    nc.sync.dma_start(out=w2f8s[:], in_=w2f8_d[:])
                    nc.sync.dma_start(out=wl1ps[:], in_=wl1p_d[:])
                    nc.scalar.dma_start(out=w1fr[:], in_=w1f_d[:])
                    nc.scalar.dma_start(out=w1bs[:], in_=w1b_d[:])
                    nc.scalar.dma_start(out=vcs[:], in_=vc_d[:])
                    nc.scalar.dma_start(out=wl2ts[:], in_=wl2t_d[:])
                    nc.scalar.dma_start(out=bl2cs[:], in_=bl2c_d[:])

                    # Gram matrix of x (incl ones column -> column sums)
                    gram_ps = startps.tile(
                        [128, D1], fp32, tag="gram", name="gram_ps"
                    )
                    for t in range(NRT):
                        o = t * D1
                        nc.tensor.matmul(
                            gram_ps[:],
                            lhsT=x2s[:, o : o + DIN],
                            rhs=x2s[:, o : o + D1],
                            start=(t == 0),
                            stop=(t == NRT - 1),
                        )
                    gram_r = startsb.tile(
                        [128, D1], f32r, tag="gram_r", name="gram_r"
                    )
                    nc.vector.tensor_copy(gram_r[:], gram_ps[:])
                    nc.scalar.mul(mean_r[:], gram_r[:, DIN : D1], 1.0 / N_TOTAL)

                    # P = G @ W1T ; V = W1T*P ; e2[f]=colsum(V) ; wxm = W1T.T mean
                    st1_ps = startps.tile(
                        [128, 16], fp32, tag="st1ps", name="st1_ps"
                    )
                    for hf in range(2):
                        sl = slice(hf * 512, (hf + 1) * 512)
                        p_ps = startps.tile(
                            [128, 512], fp32, tag=f"pps{hf}", name=f"p_ps{hf}"
                        )
                        nc.tensor.matmul(
                            p_ps[:], lhsT=gram_r[:, 0:DIN], rhs=w1fr[:, sl],
                            start=True, stop=True,
                        )
                        nc.vector.tensor_mul(v_r[:, sl], w1fr[:, sl], p_ps[:])
                    for m in range(8):
                        sl = slice(m * 128, (m + 1) * 128)
                        nc.tensor.matmul(
                            st1_ps[:, m : m + 1], lhsT=v_r[:, sl], rhs=ones_bf[:],
                            start=True, stop=True,
                        )
                        nc.tensor.matmul(
                            st1_ps[:, 8 + m : 9 + m], lhsT=w1bs[:, sl],
                            rhs=mean_r[:], start=True, stop=True,
                        )
                    nc.vector.tensor_copy(st1sb[:], st1_ps[:])
                    nc.sync.dma_start(out=cc1_in[:], in_=st1sb[:])
                    nc.gpsimd.collective_compute(
                        "AllReduce", ALU.add, replica_groups=RG,
                        ins=[cc1_in[:].opt()], outs=[cc1_out[:].opt()],
                    )
                    nc.sync.dma_start(out=st1g[:], in_=cc1_out[:])

                # -------- L1 prefill (runs during the AllReduce wait) -------
                with tc.tile_pool(name="ph1", bufs=4, space="PSUM") as ph1:
                    for ci, (c0, cc) in enumerate(CH_LIST):
                        for m in range(8):
                            hp = ph1.tile(
                                [128, CH], fp32, tag="ph1", name=f"h1_{ci}_{m}"
                            )
                            for off, nn in _halves(cc):
                                nc.tensor.matmul(
                                    hp[:, off : off + nn],
                                    lhsT=w1bs[:, m * 128 : (m + 1) * 128],
                                    rhs=xTs[:, c0 + off : c0 + off + nn],
                                    start=True,
                                    stop=True,
                                )
                            if m % 2 == 0:
                                nc.scalar.copy(
                                    hbuf[m][:, c0 : c0 + cc], hp[:, :cc]
                                )
                            else:
                                nc.vector.tensor_copy(
                                    hbuf[m][:, c0 : c0 + cc], hp[:, :cc]
                                )

            # ---------------- BN1 parameters (vectorized) ----------------
            with tc.tile_pool(name="bnw", bufs=1) as bnw:
                var8 = bnw.tile([128, 8], fp32, tag="v8", name="var8")
                msq8 = bnw.tile([128, 8], fp32, tag="m8", name="msq8")
                sd8 = bnw.tile([128, 8], fp32, tag="s8", name="sd8")
                rstd8 = bnw.tile([128, 8], fp32, tag="r8", name="rstd8")
                t8 = bnw.tile([128, 8], fp32, tag="t8", name="t8")
                nc.vector.tensor_scalar_mul(var8[:], st1g[:, 0:8], 1.0 / N_TOTAL)
                nc.vector.tensor_mul(msq8[:], st1g[:, 8:16], st1g[:, 8:16])
                nc.vector.tensor_sub(var8[:], var8[:], msq8[:])
                nc.scalar.activation(sd8[:], var8[:], AF.Sqrt, bias=eps_c[:])
                nc.vector.reciprocal(rstd8[:], sd8[:])
                nc.vector.tensor_mul(bn1_s[:], rstd8[:], vcv[:, 1, :])
                nc.vector.tensor_mul(t8[:], st1g[:, 8:16], bn1_s[:])
                nc.vector.tensor_sub(bn1_b[:], vcv[:, 2, :], t8[:])

            if stage == "s1":
                dummy = persist.tile([16, R], fp32, tag="dummy", name="dummy")
                nc.vector.memset(dummy[:], 0.0)
                nc.vector.tensor_copy(dummy[:10, 0:8], bn1_s[:10, :])
                nc.vector.tensor_copy(dummy[:10, 8:16], bn1_b[:10, :])
                nc.sync.dma_start(out=out_d[:], in_=dummy[:10, :])
                nc.finalize()
                return nc

            # ------------- main pass: BN1+ReLU -> L2 -> stats ------------
            w2bv = w2bs[:].rearrange("p (k h) -> p k h", k=6)
            w2f8v = w2f8s[:].rearrange("p (two h) -> p two h", two=2)
            with (
                tc.tile_pool(name="acts", bufs=1) as acts,
                tc.tile_pool(name="sqp", bufs=1) as sqp,
                tc.tile_pool(name="ph2", bufs=3, space="PSUM") as ph2,
            ):
                for ci, (c0, cc) in enumerate(CH_LIST):
                    sl = slice(c0, c0 + cc)
                    a1t = [
                        acts.tile(
                            [128, CH], bf16, tag=f"act{ci & 1}_{k}",
                            name=f"a1_{ci}_{k}",
                        )
                        for k in range(6)
                    ]
                    a1f8 = acts.tile(
                        [128, 2 * CH], fp8, tag=f"actf8{ci & 1}", name=f"a1f8_{ci}"
                    )
                    a1f8v = a1f8[:].rearrange("p (two c) -> p two c", two=2)
                    for m in range(8):
                        dst = (
                            a1t[m][:, :cc]
                            if m < 6
                            else a1f8[:, (m - 6) * CH : (m - 6) * CH + cc]
                        )
                        nc.scalar.activation(
                            dst,
                            hbuf[m][:, sl],
                            AF.Relu,
                            bias=bn1_b[:, m : m + 1],
                            scale=bn1_s[:, m : m + 1],
                        )
                    for m2 in range(8):
                        hp = ph2.tile([128, CH], fp32, tag="ph2", name=f"h2_{ci}_{m2}")
                        msl = slice(m2 * 128, (m2 + 1) * 128)
                        for k in range(6):
                            for off, nn in _halves(cc):
                                nc.tensor.matmul(
                                    hp[:, off : off + nn],
                                    lhsT=w2bv[:, k, msl],
                                    rhs=a1t[k][:, off : off + nn],
                                    start=(k == 0),
                                    stop=False,
                                )
                        for off, nn in _halves(cc):
                            nc.tensor.matmul(
                                hp[:, off : off + nn],
                                lhsT=w2f8v[:, :, msl],
                                rhs=a1f8v[:, :, off : off + nn],
                                start=False,
                                stop=True,
                                perf_mode=DR,
                            )
                        # PSUM -> SBUF (bf16, in place over h1) + row-sum
                        nc.vector.tensor_scalar(
                            out=hbuf[m2][:, sl],
                            in0=hp[:, :cc],
                            scalar1=1.0,
                            scalar2=0.0,
                            op0=ALU.mult,
                            op1=ALU.add,
                            accum_out=suma[:, m2 * NCH + ci : m2 * NCH + ci + 1],
                        )
                        sq = sqp.tile(
                            [128, CH], bf16, tag=f"sq{m2 & 1}", name=f"sq_{ci}_{m2}"
                        )
                        if m2 < 3:
                            nc.scalar.activation(
                                sq[:, :cc],
                                hbuf[m2][:, sl],
                                AF.Square,
                                accum_out=sumq[:, m2 * NCH + ci : m2 * NCH + ci + 1],
                            )
                        else:
                            nc.vector.scalar_tensor_tensor(
                                out=sq[:, :cc],
                                in0=hbuf[m2][:, sl],
                                scalar=1.0,
                                in1=hbuf[m2][:, sl],
                                op0=ALU.mult,
                                op1=ALU.mult,
                                accum_out=sumq[:, m2 * NCH + ci : m2 * NCH + ci + 1],
                            )

                # ---------------- BN2 statistics ----------------
                st2sb = sqp.tile([128, 16], fp32, tag="st2sb", name="st2sb")
                sumav = suma[:].rearrange("p (m c) -> p m c", m=8)
                sumqv = sumq[:].rearrange("p (m c) -> p m c", m=8)
                nc.vector.reduce_sum(st2sb[:, 0:8], sumav, axis=X)
                nc.vector.reduce_sum(st2sb[:, 8:16], sumqv, axis=X)
                nc.sync.dma_start(out=cc2_in[:], in_=st2sb[:])
                nc.gpsimd.collective_compute(
                    "AllReduce", ALU.add, replica_groups=RG,
                    ins=[cc2_in[:].opt()], outs=[cc2_out[:].opt()],
                )
                nc.sync.dma_start(out=st2g[:], in_=cc2_out[:])

            # ---------------- BN2 parameters (vectorized) ----------------
            with tc.tile_pool(name="bnw2", bufs=1) as bnw2:
                mean8 = bnw2.tile([128, 8], fp32, tag="n8", name="mean8")
                var8b = bnw2.tile([128, 8], fp32, tag="v8", name="var8b")
                msq8b = bnw2.tile([128, 8], fp32, tag="m8", name="msq8b")
                sd8b = bnw2.tile([128, 8], fp32, tag="s8", name="sd8b")
                rstd8b = bnw2.tile([128, 8], fp32, tag="r8", name="rstd8b")
                t8b = bnw2.tile([128, 8], fp32, tag="t8", name="t8b")
                nc.vector.tensor_scalar_mul(mean8[:], st2g[:, 0:8], 1.0 / N_TOTAL)
                nc.vector.tensor_mul(msq8b[:], mean8[:], mean8[:])
                nc.vector.scalar_tensor_tensor(
                    out=var8b[:], in0=st2g[:, 8:16], scalar=1.0 / N_TOTAL,
                    in1=msq8b[:], op0=ALU.mult, op1=ALU.subtract,
                )
                nc.scalar.activation(sd8b[:], var8b[:], AF.Sqrt, bias=eps_c[:])
                nc.vector.reciprocal(rstd8b[:], sd8b[:])
                nc.vector.tensor_mul(bn2_s[:], rstd8b[:], vcv[:, 4, :])
                nc.vector.tensor_mul(t8b[:], mean8[:], bn2_s[:])
                nc.vector.tensor_sub(bn2_b[:], vcv[:, 5, :], t8b[:])

            # ------ final pass A: BN2+ReLU -> L3 -> L4 -> logits ------
            wl1v = wl1ps[:].rearrange("p (kp two h) -> p kp two h", kp=4, two=2)
            with (
                tc.tile_pool(name="acts2", bufs=1) as acts2,
                tc.tile_pool(name="fsb", bufs=2) as fsb,
                tc.tile_pool(name="ph3", bufs=3, space="PSUM") as ph3,
                tc.tile_pool(name="plg", bufs=2, space="PSUM") as plg,
            ):
                for ci, (c0, cc) in enumerate(FCH_LIST):
                    sl = slice(c0, c0 + cc)
                    a2t = [
                        acts2.tile(
                            [128, FCH], l3dt, tag=f"act{ci & 1}_{k}",
                            name=f"a2_{ci}_{k}",
                        )
                        for k in range(8)
                    ]
                    for m in range(8):
                        if m < 5:
                            nc.scalar.activation(
                                a2t[m][:, :cc],
                                hbuf[m][:, sl],
                                AF.Relu,
                                bias=bn2_b[:, m : m + 1],
                                scale=bn2_s[:, m : m + 1],
                            )
                        else:
                            tmp = fsb.tile(
                                [128, FCH], bf16, tag=f"tmp{m}", name=f"tmp_{ci}_{m}"
                            )
                            nc.vector.tensor_scalar(
                                out=tmp[:, :cc],
                                in0=hbuf[m][:, sl],
                                scalar1=bn2_s[:, m : m + 1],
                                scalar2=bn2_b[:, m : m + 1],
                                op0=ALU.mult,
                                op1=ALU.add,
                            )
                            nc.vector.tensor_scalar_max(
                                a2t[m][:, :cc], tmp[:, :cc], 0.0
                            )
                    h3t = []
                    for m3 in range(2):
                        hp3 = ph3.tile(
                            [128, FCH], fp32, tag="ph3", name=f"h3_{ci}_{m3}"
                        )
                        msl = slice(m3 * 128, (m3 + 1) * 128)
                        for k in range(8):
                            nc.tensor.matmul(
                                hp3[:, :cc],
                                lhsT=wl1v[:, k // 2, k % 2, msl],
                                rhs=a2t[k][:, :cc],
                                start=(k == 0),
                                stop=(k == 7),
                            )
                        h3 = fsb.tile(
                            [128, FCH], bf16, tag=f"h3_{m3}", name=f"h3t_{ci}_{m3}"
                        )
                        nc.vector.tensor_scalar(
                            out=h3[:, :cc],
                            in0=hp3[:, :cc],
                            scalar1=vcv[:, 6, m3 : m3 + 1],
                            scalar2=0.0,
                            op0=ALU.add,
                            op1=ALU.max,
                        )
                        h3t.append(h3)
                    # L4: logits (pre-bl2) -> PSUM -> lgall (bf16)
                    lg = plg.tile([C, FCH], fp32, tag="lg", name=f"lg_{ci}")
                    for k in range(2):
                        nc.tensor.matmul(
                            lg[:, :cc],
                            lhsT=wl2ts[:, k * C : (k + 1) * C],
                            rhs=h3t[k][:, :cc],
                            start=(k == 0),
                            stop=(k == 1),
                        )
                    nc.vector.tensor_scalar(
                        out=lgall[:C, sl],
                        in0=lg[:, :cc],
                        scalar1=bl2cs[:C, :],
                        scalar2=0.0,
                        op0=ALU.add,
                        op1=ALU.add,
                    )

            # ---------- batched log_softmax over [10, R] ----------
            with (
                tc.tile_pool(name="smx", bufs=2) as smx,
                tc.tile_pool(name="psmB", bufs=2, space="PSUM") as psmB,
            ):
                EW = 2048
                for o0 in range(0, R, EW):
                    ee = min(EW, R - o0)
                    nc.scalar.activation(
                        e_all[:C, o0 : o0 + ee],
                        lgall[:C, o0 : o0 + ee],
                        AF.Exp,
                    )
                NF = len(FCH_LIST)
                lns_t = {}
                for step in range(NF + 1):
                    if step < NF:
                        c0, cc = FCH_LIST[step]
                        sm = psmB.tile(
                            [1, FCH], fp32, tag=f"sm{step & 1}", name=f"smB_{step}"
                        )
                        nc.tensor.matmul(
                            sm[:, :cc],
                            lhsT=ones_bf[:C, :],
                            rhs=e_all[:C, c0 : c0 + cc],
                            start=True,
                            stop=True,
                        )
                        lns = smx.tile(
                            [1, FCH], bf16, tag=f"lns{step & 1}", name=f"lnsB_{step}"
                        )
                        nc.scalar.activation(lns[:, :cc], sm[:, :cc], AF.Ln)
                        lns_t[step] = lns
                    if step >= 1:
                        ci = step - 1
                        c0, cc = FCH_LIST[ci]
                        sl = slice(c0, c0 + cc)
                        bc = psmB.tile(
                            [C, FCH], fp32, tag=f"bc{ci & 1}", name=f"bcB_{ci}"
                        )
                        nc.tensor.matmul(
                            bc[:, :cc],
                            lhsT=mones_bf[:, :C],
                            rhs=lns_t[ci][:, :cc],
                            start=True,
                            stop=True,
                        )
                        osb = smx.tile(
                            [C, FCH], fp32, tag=f"osb{ci & 1}", name=f"osbB_{ci}"
                        )
                        nc.vector.tensor_add(
                            osb[:, :cc], lgall[:C, sl], bc[:, :cc]
                        )
                        nc.sync.dma_start(out=out_d[:, sl], in_=osb[:, :cc])

    nc.finalize()
    return nc


def _get_nc():
    if "nc" not in _CACHE:
        _CACHE["nc"] = _build(os.environ.get("KERNEL_STAGE", "full"))
    return _CACHE["nc"]


def make_in_maps(inputs):
    """Host-side layout/dtype prep. Layout-only transforms (transpose, tile,
    pack, cast) - all math happens on device."""
    f32 = np.float32
    bf = ml_dtypes.bfloat16
    f8 = ml_dtypes.float8_e4m3

    x = np.ascontiguousarray(np.asarray(inputs["x"]), dtype=f32)
    W1 = np.asarray(inputs["W1"], dtype=f32)
    W2 = np.asarray(inputs["W2"], dtype=f32)
    Wl1 = np.asarray(inputs["Wl1"], dtype=f32)
    Wl2 = np.asarray(inputs["Wl2"], dtype=f32)

    w1f = np.ascontiguousarray(W1.T)  # [128, H] f32 (f32r on device)
    w1b = w1f.astype(bf)
    l3np = f8 if L3_FP8 else bf
    # L2: first 768 contraction dims bf16, last 256 packed fp8 (DoubleRow)
    w2b = np.ascontiguousarray(
        W2.T[:768].reshape(6, 128, H).transpose(1, 0, 2).reshape(128, 6 * H)
    ).astype(bf)
    w2f8 = np.ascontiguousarray(
        W2.T[768:].reshape(2, 128, H).transpose(1, 0, 2).reshape(128, 2 * H)
    ).astype(f8)
    wl1p = np.ascontiguousarray(
        Wl1.T.reshape(4, 2, 128, HM).transpose(2, 0, 1, 3).reshape(128, 8 * HM)
    ).astype(l3np)
    wl2t = np.ascontiguousarray(
        Wl2.T.reshape(2, 128, C).transpose(1, 0, 2).reshape(128, 2 * C)
    ).astype(bf)

    vecs = np.zeros((8, H), f32)
    vecs[1, :] = np.asarray(inputs["g1"], dtype=f32)
    vecs[2, :] = np.asarray(inputs["be1"], dtype=f32)
    vecs[4, :] = np.asarray(inputs["g2"], dtype=f32)
    vecs[5, :] = np.asarray(inputs["be2"], dtype=f32)
    vecs[6, :HM] = np.asarray(inputs["bl1"], dtype=f32)
    # vc[p, k, j] = vecs[j, k*128+p]
    vc = np.ascontiguousarray(
        vecs.T.reshape(8, 128, 8).transpose(1, 0, 2).reshape(128, 64)
    )
    bl2c = np.zeros((16, 1), f32)
    bl2c[:C, 0] = np.asarray(inputs["bl2"], dtype=f32)

    in_maps = []
    for i in range(NCORES):
        xs = x[i * R : (i + 1) * R]
        xt = np.ascontiguousarray(xs.T).astype(bf)
        x2 = np.zeros((NRT * 128, D1), f32)
        x2[:R, :DIN] = xs
        x2[:R, DIN] = 1.0
        x2 = np.ascontiguousarray(
            x2.reshape(NRT, 128, D1).transpose(1, 0, 2).reshape(128, NRT * D1)
        ).astype(bf)
        in_maps.append(
            {
                "x2": x2, "xT": xt, "w1f": w1f, "w1b": w1b,
                "w2b": w2b, "w2f8": w2f8,
                "wl1p": wl1p, "wl2t": wl2t, "vc": vc, "bl2c": bl2c,
                "ones": np.ones((128, 1), f32),
                "mones": np.full((1, 16), -1.0, f32),
            }
        )
    return in_maps


def kernel(**inputs):
    from concourse.bass_utils import run_bass_kernel_spmd

    nc = _get_nc()
    in_maps = make_in_maps(inputs)
    res = run_bass_kernel_spmd(nc, in_maps, core_ids=list(range(NCORES)))
    return np.concatenate(
        [np.asarray(r["out"], dtype=np.float32).T for r in res.results], axis=0
    )
